# revision 9
# baseline (speedup 1.0000x reference)
"""Causal attention (B=8, S=2048, D=H=768) on 8 trn2 NeuronCores.

Data-parallel over batch: core c computes batch c entirely on-chip, no
collectives.  All matmuls contract over the partition dim.

Key algebraic move: scores = (x Wq)(x Wk)^T = x (Wq Wk^T) x^T, with
M = Wq Wk^T precomputed on host (768x768).  That folds the q AND k
projections into ONE on-device projection t = x M, and the scores'
k-side operand becomes raw x^T.

Precision scheme (HW-validated):
  - V = x Wv and t = x M run in single-pass fp16 (operand rounding only;
    PE accumulates exactly in fp32 PSUM).  t is stored as fp32 (float32r).
  - scores = t x^T runs as a SINGLE-PASS float32r matmul: fp32 operands
    in SBUF, ~2^-13.5 internal product truncation, 1.5 PE cycles/row --
    kills both the fp16 store-rounding of t and the k-side x rounding.
  - exp weights, transposes, and attn@V run in fp16; softmax stats fp32.

Per-core pipeline:
  phase 1b: V[s,h] = x16-blocks (stationary) x Wv16 (moving), fp16.
    The two head-critical DMAs (wv half 0, x16 chunk 0) are triggered
    from the Tensor queue, which is idle at t=0 (the Sync queue spends
    ~8us on semaphore init first).  Bulk loads (x32 chunks, m16) ride
    the SWDGE ring gated behind the first V copies.
  phase 1a: tT = M16 (stationary) x x16T (moving), fp16 -> f32r store.
    s-chunks run REVERSED (3,2,1,0) so the first phase-2 tiles (high qt)
    see their t chunk earliest.
  phase 2, per 128-row q-tile, qt descending 15..0: scores strip
    [q, k<=q] f32r; one DVE op per 512-chunk computes
    strip = causal_mask - psum (masked slots +1e10); a min-reduce gives
    -rowmax; exp on ScalarE (scale=-1, bias=-max, accum_out=rowsum)
    -> fp16; PE-transpose exp in batches of 4 blocks
    per PSUM tile with one copy each; out = sum_k expT x V (fp16);
    scale by 1/rowsum into an fp16 accumulator; one DMA per FOUR
    q-tiles ([128, 4*768] f16, partition-major DRAM layout) keeps the
    per-line descriptor overhead amortized and the end-of-kernel DMA
    drain short.

Host side: shards x over batch, pre-transposes/tiles, computes
M = Wq Wk^T in float64, replicates weights, gathers + de-tiles outputs.
"""

from contextlib import ExitStack

import numpy as np

import bass_rust
import concourse.mybir as mybir
import concourse.tile as tile
from concourse import bacc
from concourse.bass_utils import run_bass_kernel_spmd
from concourse.masks import make_causal_mask, make_identity

B, S, D, H = 8, 2048, 768, 768
N_CORES = 8
P = 128
DT = D // P    # 6 d-tiles
HT = H // P    # 6 h-tiles
ST = S // P    # 16 s-tiles
SC = S // 512  # 4 column-chunks

f32 = mybir.dt.float32
f32r = mybir.dt.float32r
f16 = mybir.dt.float16

# "mixed": scores in f32r (x32 shipped), t stored f32r.  rel err ~9e-3.
# "f16":   scores in fp16 (no x32), t stored f16.        rel err ~1.25e-2.
SCHEME = "mixed"


def _ceil_div(a, b):
    return (a + b - 1) // b


def build_nc(scheme=SCHEME):
    mixed = scheme == "mixed"
    nc = bacc.Bacc(None)

    # inputs ship pre-tiled from the host in exact SBUF layout
    # ([128 partitions, ...]) so every DMA line is fully contiguous
    x16_d = nc.declare_dram_parameter("x16", [SC, P, DT, 512], f16, isOutput=False)
    m16_d = nc.declare_dram_parameter("m16", [P, DT, D], f16, isOutput=False)
    wv_d = nc.declare_dram_parameter("wv", [2, P, DT, 384], f16, isOutput=False)
    if mixed:
        x32_d = nc.declare_dram_parameter(
            "x32", [SC, P, DT, 512], f32r, isOutput=False)
    # partition-major output: host de-tiles [P, ST, H] -> [S, H]
    out_d = nc.declare_dram_parameter("out", [P, ST, H], f16, isOutput=True)

    t_dt = f32r if mixed else f16

    with tile.TileContext(nc, pool_alloc_mode="queue") as tc, ExitStack() as ctx:
        persist = ctx.enter_context(tc.tile_pool(name="persist", bufs=1))
        t_s = persist.tile([P, HT, S], t_dt)     # 48KB/part (24 if f16)
        V = persist.tile([P, ST, H], f16)        # 24KB/part
        ident16 = persist.tile([P, P], f16)
        # zcm = [512 zeros | 128-col causal mask]; slice [640-w:640] puts
        # the mask on the last 128 of a w-wide diag chunk, zeros elsewhere
        zcm = persist.tile([P, 640], f32)
        if mixed:
            x32_s = persist.tile([P, SC, DT, 512], f32r)  # 48KB/part

        p1pool = tc.alloc_tile_pool(name="p1", bufs=1)
        x16_s = p1pool.tile([P, SC, DT, 512], f16)  # 24KB/part
        m16_s = p1pool.tile([P, DT, D], f16)
        wv_s = p1pool.tile([P, 2, DT, 384], f16)

        # head-critical loads on the GpSimd queue, whose preamble clears
        # ~5us before the Sync queue's: first V group needs exactly these
        nc.gpsimd.dma_start(out=wv_s[:, 0], in_=wv_d[0])
        nc.gpsimd.dma_start(out=x16_s[:, 0], in_=x16_d[0])
        # near-term loads on the Sync queue
        nc.sync.dma_start(out=wv_s[:, 1], in_=wv_d[1])
        for sc in range(1, SC):
            nc.sync.dma_start(out=x16_s[:, sc], in_=x16_d[sc])

        make_identity(nc, ident16)
        nc.gpsimd.memset(zcm[:, 0:512], 0.0)
        make_causal_mask(nc, zcm[:, 512:640], mask_val=1e10)

        # ---- phase 1b: V = x16 (stationary) x Wv16 (moving) --------------
        with tc.tile_pool(name="p1b_ps", bufs=4, space="PSUM") as pp:
            bulk_anchor = None
            for sc in range(SC):
                if sc == 1:
                    # bulk loads (x32 24KB/part + m16) stream on the SWDGE
                    # ring once the head-critical transfers are done
                    assert bulk_anchor is not None
                    bulk = [(m16_s, m16_d[:, :, :])]
                    if mixed:
                        bulk += [(x32_s[:, c], x32_d[c]) for c in range(SC)]
                    for dst, src in bulk:
                        dma = nc.gpsimd.dma_start(out=dst, in_=src)
                        bass_rust.add_dep_helper(
                            dma.ins, bulk_anchor.ins, sync=True,
                            reason="bulk load waits for first V chunk")
                for hc in range(2):
                    for sti in range(4):
                        off = sti * P
                        ps = pp.tile([P, 384], f32, tag="psv", name="psv")
                        for dt_ in range(DT):
                            nc.tensor.matmul(
                                ps,
                                x16_s[:, sc, dt_, off:off + P],
                                wv_s[:, hc, dt_, :],
                                start=(dt_ == 0),
                                stop=(dt_ == DT - 1),
                            )
                        cp = nc.vector.tensor_copy(
                            V[:, sc * 4 + sti, hc * 384:(hc + 1) * 384], ps)
                        if sc == 0 and hc == 0 and sti == 3:
                            bulk_anchor = cp

        # ---- phase 1a: tT = M16 (stationary) x x16T (moving) -------------
        # reversed s-chunk order: phase 2 runs qt descending, so high-qt
        # tiles (which need the last t chunk for their stationary) unblock
        # right after the first chunk-group here
        with tc.tile_pool(name="p1a_ps", bufs=4, space="PSUM") as pp:
            for sc in range(SC - 1, -1, -1):
                for ht in range(HT):
                    ps = pp.tile([P, 512], f32, tag="ps", name="ps")
                    for dt_ in range(DT):
                        nc.tensor.matmul(
                            ps,
                            m16_s[:, dt_, ht * P:(ht + 1) * P],
                            x16_s[:, sc, dt_, :],
                            start=(dt_ == 0),
                            stop=(dt_ == DT - 1),
                        )
                    nc.scalar.copy(t_s[:, ht, sc * 512:(sc + 1) * 512], ps)
        if mixed:
            p1pool.release()
            xk_s = x32_s       # scores k-side operand
        else:
            xk_s = x16_s       # fp16 scores read x16 directly (persists)

        # ---- phase 2: attention ------------------------------------------
        with tc.tile_pool(name="p2_strip", bufs=3) as strip_pool, \
             tc.tile_pool(name="p2_exp", bufs=3) as exp_pool, \
             tc.tile_pool(name="p2_expT", bufs=3) as expT_pool, \
             tc.tile_pool(name="p2_stat", bufs=6) as stat_pool, \
             tc.tile_pool(name="p2_acc", bufs=2) as acc_pool, \
             tc.tile_pool(name="p2_ps_s", bufs=4, space="PSUM") as ps_s_pool, \
             tc.tile_pool(name="p2_ps_t", bufs=2, space="PSUM") as ps_t_pool, \
             tc.tile_pool(name="p2_ps_o", bufs=2, space="PSUM") as ps_o_pool:
            acc = None
            for qt in range(ST - 1, -1, -1):
                L = qt + 1
                cols = L * P
                nchn = _ceil_div(cols, 512)
                strip = strip_pool.tile([P, S], f32, tag="strip", name="strip")
                for nch in range(nchn):
                    w = min(512, cols - nch * 512)
                    # f32r needs moving >= 256 for full rate; pad short
                    # tails with throwaway columns
                    wp = max(w, 256) if mixed else w
                    ps = ps_s_pool.tile([P, 512], f32, tag="ps_s", name="ps_s")
                    for dt_ in range(DT):
                        nc.tensor.matmul(
                            ps[:, :wp],
                            t_s[:, dt_, qt * P:(qt + 1) * P],
                            xk_s[:, nch, dt_, 0:wp],
                            start=(dt_ == 0),
                            stop=(dt_ == DT - 1),
                        )
                    # strip = mask - scores (masked slots become +1e10,
                    # transparent to the min-reduce below)
                    in1 = zcm[:, 640 - w:640] if nch == nchn - 1 else zcm[:, 0:w]
                    nc.vector.tensor_sub(
                        strip[:, nch * 512:nch * 512 + w], in1, ps[:, :w])
                nmax = stat_pool.tile([P, 1], f32, tag="nmax", name="nmax")
                nc.vector.tensor_reduce(
                    nmax, strip[:, :cols],
                    axis=mybir.AxisListType.X, op=mybir.AluOpType.min,
                )
                rsum = stat_pool.tile([P, 1], f32, tag="rsum", name="rsum")
                exp16 = exp_pool.tile([P, S], f16, tag="exp16", name="exp16")
                # exp(in*-1 + (-max)) = exp(scores + mask - max)
                nc.scalar.activation(
                    exp16[:, :cols], strip[:, :cols],
                    mybir.ActivationFunctionType.Exp,
                    bias=nmax, scale=-1.0, accum_out=rsum,
                )
                rinv = stat_pool.tile([P, 1], f32, tag="rinv", name="rinv")
                nc.vector.reciprocal(rinv, rsum)
                expT = expT_pool.tile([P, ST, P], f16, tag="expT", name="expT")
                for j0 in range(0, L, 4):
                    jn = min(4, L - j0)
                    pst = ps_t_pool.tile([P, 512], f16, tag="ps_t", name="ps_t")
                    for i in range(jn):
                        nc.tensor.transpose(
                            pst[:, i * P:(i + 1) * P],
                            exp16[:, (j0 + i) * P:(j0 + i + 1) * P],
                            ident16)
                    nc.scalar.copy(expT[:, j0:j0 + jn, :], pst[:, :jn * P])
                if qt % 4 == 3:
                    acc = acc_pool.tile([P, 4, H], f16, tag="acc", name="acc")
                for hc in range(2):
                    pso = ps_o_pool.tile([P, 384], f32, tag="ps_o", name="ps_o")
                    for j in range(L):
                        nc.tensor.matmul(
                            pso,
                            expT[:, j, :],
                            V[:, j, hc * 384:(hc + 1) * 384],
                            start=(j == 0),
                            stop=(j == L - 1),
                        )
                    nc.vector.tensor_scalar_mul(
                        acc[:, qt % 4, hc * 384:(hc + 1) * 384], pso, rinv)
                if qt % 4 == 0:
                    nc.sync.dma_start(
                        out=out_d[:, qt:qt + 4, :], in_=acc)

    nc.finalize()
    return nc


_NC_CACHE = None


def _get_nc():
    global _NC_CACHE
    if _NC_CACHE is None:
        _NC_CACHE = build_nc()
    return _NC_CACHE


def _tile_rows(a):
    """[D, N] -> [128, D//128, N] (partition-major SBUF layout)."""
    d, n = a.shape
    return np.ascontiguousarray(a.reshape(d // P, P, n).transpose(1, 0, 2))


def make_in_maps(x, Wq, Wk, Wv):
    M = (Wq.astype(np.float64) @ Wk.astype(np.float64).T).astype(np.float32)
    m16 = _tile_rows(M.astype(np.float16))
    wv16 = _tile_rows(Wv.astype(np.float16))
    wv16 = np.ascontiguousarray(
        wv16.reshape(P, DT, 2, 384).transpose(2, 0, 1, 3))
    mixed = SCHEME == "mixed"
    in_maps = []
    for c in range(N_CORES):
        xT = np.ascontiguousarray(x[c].T)  # [D, S] f32
        xc32 = np.ascontiguousarray(
            xT.reshape(DT, P, SC, 512).transpose(2, 1, 0, 3))
        im = {"x16": xc32.astype(np.float16), "m16": m16, "wv": wv16}
        if mixed:
            im["x32"] = xc32
        in_maps.append(im)
    return in_maps


def kernel(x, Wq, Wk, Wv):
    x = np.asarray(x, dtype=np.float32)
    Wq = np.asarray(Wq, dtype=np.float32)
    Wk = np.asarray(Wk, dtype=np.float32)
    Wv = np.asarray(Wv, dtype=np.float32)

    nc = _get_nc()
    in_maps = make_in_maps(x, Wq, Wk, Wv)
    res = run_bass_kernel_spmd(nc, in_maps, list(range(N_CORES)))
    out = np.stack(
        [res.results[c]["out"].transpose(1, 0, 2).reshape(S, H)
         for c in range(N_CORES)], axis=0)
    return out.astype(np.float32)


# revision 10
# speedup vs baseline: 1.0181x; 1.0181x over previous
"""Causal attention (B=8, S=2048, D=H=768) on 8 trn2 NeuronCores.

Data-parallel over batch: core c computes batch c entirely on-chip, no
collectives.  All matmuls contract over the partition dim.

Key algebraic move: scores = (x Wq)(x Wk)^T = x (Wq Wk^T) x^T, with
M = Wq Wk^T precomputed on host (768x768).  That folds the q AND k
projections into ONE on-device projection t = x M, and the scores'
k-side operand becomes raw x^T.

Precision scheme (HW-validated):
  - V = x Wv and t = x M run in single-pass fp16 (operand rounding only;
    PE accumulates exactly in fp32 PSUM).  t is stored as fp32 (float32r).
  - scores = t x^T runs as a SINGLE-PASS float32r matmul: fp32 operands
    in SBUF, ~2^-13.5 internal product truncation, 1.5 PE cycles/row --
    kills both the fp16 store-rounding of t and the k-side x rounding.
  - exp weights, transposes, and attn@V run in fp16; softmax stats fp32.

Per-core pipeline:
  phase 1b: V[s,h] = x16-blocks (stationary) x Wv16 (moving), fp16.
    The two head-critical DMAs (wv half 0, x16 chunk 0) are triggered
    from the Tensor queue, which is idle at t=0 (the Sync queue spends
    ~8us on semaphore init first).  Bulk loads (x32 chunks, m16) ride
    the SWDGE ring gated behind the first V copies.
  phase 1a: tT = M16 (stationary) x x16T (moving), fp16 -> f32r store.
    s-chunks run REVERSED (3,2,1,0) so the first phase-2 tiles (high qt)
    see their t chunk earliest.
  phase 2, per 128-row q-tile, qt descending 15..0: scores strip
    [q, k<=q] f32r; one DVE op per 512-chunk computes
    strip = causal_mask - psum (masked slots +1e10); a min-reduce gives
    -rowmax; exp on ScalarE (scale=-1, bias=-max, accum_out=rowsum)
    -> fp16; PE-transpose exp in batches of 4 blocks
    per PSUM tile with one copy each; out = sum_k expT x V (fp16);
    scale by 1/rowsum into an fp16 accumulator; one DMA per FOUR
    q-tiles ([128, 4*768] f16, partition-major DRAM layout) keeps the
    per-line descriptor overhead amortized and the end-of-kernel DMA
    drain short.

Host side: shards x over batch, pre-transposes/tiles, computes
M = Wq Wk^T in float64, replicates weights, gathers + de-tiles outputs.
"""

from contextlib import ExitStack

import numpy as np

import bass_rust
import concourse.mybir as mybir
import concourse.tile as tile
from concourse import bacc
from concourse.bass_utils import run_bass_kernel_spmd
from concourse.masks import make_causal_mask, make_identity

B, S, D, H = 8, 2048, 768, 768
N_CORES = 8
P = 128
DT = D // P    # 6 d-tiles
HT = H // P    # 6 h-tiles
ST = S // P    # 16 s-tiles
SC = S // 512  # 4 column-chunks

f32 = mybir.dt.float32
f32r = mybir.dt.float32r
f16 = mybir.dt.float16

# "mixed": scores in f32r (x32 shipped), t stored f32r.  rel err ~9e-3.
# "f16":   scores in fp16 (no x32), t stored f16.        rel err ~1.25e-2.
SCHEME = "mixed"


def _ceil_div(a, b):
    return (a + b - 1) // b


def build_nc(scheme=SCHEME):
    mixed = scheme == "mixed"
    nc = bacc.Bacc(None)

    # inputs ship pre-tiled from the host in exact SBUF layout
    # ([128 partitions, ...]) so every DMA line is fully contiguous
    x16_d = nc.declare_dram_parameter("x16", [SC, P, DT, 512], f16, isOutput=False)
    m16_d = nc.declare_dram_parameter("m16", [P, DT, D], f16, isOutput=False)
    wv_d = nc.declare_dram_parameter("wv", [2, P, DT, 384], f16, isOutput=False)
    if mixed:
        x32_d = nc.declare_dram_parameter(
            "x32", [SC, P, DT, 512], f32r, isOutput=False)
    # partition-major output: host de-tiles [P, ST, H] -> [S, H]
    out_d = nc.declare_dram_parameter("out", [P, ST, H], f16, isOutput=True)

    t_dt = f32r if mixed else f16

    with tile.TileContext(nc, pool_alloc_mode="queue") as tc, ExitStack() as ctx:
        persist = ctx.enter_context(tc.tile_pool(name="persist", bufs=1))
        t_s = persist.tile([P, HT, S], t_dt)     # 48KB/part (24 if f16)
        V = persist.tile([P, ST, H], f16)        # 24KB/part
        ident16 = persist.tile([P, P], f16)
        # zcm = [512 zeros | 128-col causal mask]; slice [640-w:640] puts
        # the mask on the last 128 of a w-wide diag chunk, zeros elsewhere
        zcm = persist.tile([P, 640], f32)
        if mixed:
            x32_s = persist.tile([P, SC, DT, 512], f32r)  # 48KB/part

        p1pool = tc.alloc_tile_pool(name="p1", bufs=1)
        x16_s = p1pool.tile([P, SC, DT, 512], f16)  # 24KB/part
        m16_s = p1pool.tile([P, DT, D], f16)
        wv_s = p1pool.tile([P, 2, DT, 384], f16)

        # head-critical loads on the Activation queue (its preamble clears
        # ~4us before the Sync queue's): first V group needs exactly these
        nc.scalar.dma_start(out=wv_s[:, 0], in_=wv_d[0])
        nc.scalar.dma_start(out=x16_s[:, 0], in_=x16_d[0])
        # near-term loads on the Sync queue
        nc.sync.dma_start(out=wv_s[:, 1], in_=wv_d[1])
        for sc in range(1, SC):
            nc.sync.dma_start(out=x16_s[:, sc], in_=x16_d[sc])

        make_identity(nc, ident16)
        nc.gpsimd.memset(zcm[:, 0:512], 0.0)
        make_causal_mask(nc, zcm[:, 512:640], mask_val=1e10)

        # ---- phase 1b: V = x16 (stationary) x Wv16 (moving) --------------
        with tc.tile_pool(name="p1b_ps", bufs=4, space="PSUM") as pp:
            bulk_anchor = None
            for sc in range(SC):
                if sc == 1:
                    # bulk loads (x32 24KB/part + m16) stream on the SWDGE
                    # ring once the head-critical transfers are done
                    assert bulk_anchor is not None
                    bulk = [(m16_s, m16_d[:, :, :])]
                    if mixed:
                        bulk += [(x32_s[:, c], x32_d[c]) for c in range(SC)]
                    for dst, src in bulk:
                        dma = nc.gpsimd.dma_start(out=dst, in_=src)
                        bass_rust.add_dep_helper(
                            dma.ins, bulk_anchor.ins, sync=True,
                            reason="bulk load waits for first V chunk")
                for hc in range(2):
                    for sti in range(4):
                        off = sti * P
                        ps = pp.tile([P, 384], f32, tag="psv", name="psv")
                        for dt_ in range(DT):
                            nc.tensor.matmul(
                                ps,
                                x16_s[:, sc, dt_, off:off + P],
                                wv_s[:, hc, dt_, :],
                                start=(dt_ == 0),
                                stop=(dt_ == DT - 1),
                            )
                        cp = nc.vector.tensor_copy(
                            V[:, sc * 4 + sti, hc * 384:(hc + 1) * 384], ps)
                        if sc == 0 and hc == 0 and sti == 3:
                            bulk_anchor = cp

        # ---- phase 1a: tT = M16 (stationary) x x16T (moving) -------------
        # reversed s-chunk order: phase 2 runs qt descending, so high-qt
        # tiles (which need the last t chunk for their stationary) unblock
        # right after the first chunk-group here
        with tc.tile_pool(name="p1a_ps", bufs=4, space="PSUM") as pp:
            for sc in range(SC - 1, -1, -1):
                for ht in range(HT):
                    ps = pp.tile([P, 512], f32, tag="ps", name="ps")
                    for dt_ in range(DT):
                        nc.tensor.matmul(
                            ps,
                            m16_s[:, dt_, ht * P:(ht + 1) * P],
                            x16_s[:, sc, dt_, :],
                            start=(dt_ == 0),
                            stop=(dt_ == DT - 1),
                        )
                    nc.scalar.copy(t_s[:, ht, sc * 512:(sc + 1) * 512], ps)
        if mixed:
            p1pool.release()
            xk_s = x32_s       # scores k-side operand
        else:
            xk_s = x16_s       # fp16 scores read x16 directly (persists)

        # ---- phase 2: attention ------------------------------------------
        with tc.tile_pool(name="p2_strip", bufs=3) as strip_pool, \
             tc.tile_pool(name="p2_exp", bufs=3) as exp_pool, \
             tc.tile_pool(name="p2_expT", bufs=3) as expT_pool, \
             tc.tile_pool(name="p2_stat", bufs=6) as stat_pool, \
             tc.tile_pool(name="p2_acc", bufs=2) as acc_pool, \
             tc.tile_pool(name="p2_ps_s", bufs=4, space="PSUM") as ps_s_pool, \
             tc.tile_pool(name="p2_ps_t", bufs=2, space="PSUM") as ps_t_pool, \
             tc.tile_pool(name="p2_ps_o", bufs=2, space="PSUM") as ps_o_pool:
            acc = None
            for qt in range(ST - 1, -1, -1):
                L = qt + 1
                cols = L * P
                nchn = _ceil_div(cols, 512)
                strip = strip_pool.tile([P, S], f32, tag="strip", name="strip")
                for nch in range(nchn):
                    w = min(512, cols - nch * 512)
                    # f32r needs moving >= 256 for full rate; pad short
                    # tails with throwaway columns
                    wp = max(w, 256) if mixed else w
                    ps = ps_s_pool.tile([P, 512], f32, tag="ps_s", name="ps_s")
                    for dt_ in range(DT):
                        nc.tensor.matmul(
                            ps[:, :wp],
                            t_s[:, dt_, qt * P:(qt + 1) * P],
                            xk_s[:, nch, dt_, 0:wp],
                            start=(dt_ == 0),
                            stop=(dt_ == DT - 1),
                        )
                    # strip = mask - scores (masked slots become +1e10,
                    # transparent to the min-reduce below)
                    in1 = zcm[:, 640 - w:640] if nch == nchn - 1 else zcm[:, 0:w]
                    nc.vector.tensor_sub(
                        strip[:, nch * 512:nch * 512 + w], in1, ps[:, :w])
                nmax = stat_pool.tile([P, 1], f32, tag="nmax", name="nmax")
                nc.vector.tensor_reduce(
                    nmax, strip[:, :cols],
                    axis=mybir.AxisListType.X, op=mybir.AluOpType.min,
                )
                rsum = stat_pool.tile([P, 1], f32, tag="rsum", name="rsum")
                exp16 = exp_pool.tile([P, S], f16, tag="exp16", name="exp16")
                # exp(in*-1 + (-max)) = exp(scores + mask - max)
                nc.scalar.activation(
                    exp16[:, :cols], strip[:, :cols],
                    mybir.ActivationFunctionType.Exp,
                    bias=nmax, scale=-1.0, accum_out=rsum,
                )
                rinv = stat_pool.tile([P, 1], f32, tag="rinv", name="rinv")
                nc.vector.reciprocal(rinv, rsum)
                expT = expT_pool.tile([P, ST, P], f16, tag="expT", name="expT")
                for j0 in range(0, L, 4):
                    jn = min(4, L - j0)
                    pst = ps_t_pool.tile([P, 512], f16, tag="ps_t", name="ps_t")
                    for i in range(jn):
                        nc.tensor.transpose(
                            pst[:, i * P:(i + 1) * P],
                            exp16[:, (j0 + i) * P:(j0 + i + 1) * P],
                            ident16)
                    nc.scalar.copy(expT[:, j0:j0 + jn, :], pst[:, :jn * P])
                if qt % 4 == 3:
                    acc = acc_pool.tile([P, 4, H], f16, tag="acc", name="acc")
                for hc in range(2):
                    pso = ps_o_pool.tile([P, 384], f32, tag="ps_o", name="ps_o")
                    for j in range(L):
                        nc.tensor.matmul(
                            pso,
                            expT[:, j, :],
                            V[:, j, hc * 384:(hc + 1) * 384],
                            start=(j == 0),
                            stop=(j == L - 1),
                        )
                    nc.vector.tensor_scalar_mul(
                        acc[:, qt % 4, hc * 384:(hc + 1) * 384], pso, rinv)
                if qt % 4 == 0:
                    nc.sync.dma_start(
                        out=out_d[:, qt:qt + 4, :], in_=acc)

    nc.finalize()
    return nc


_NC_CACHE = None


def _get_nc():
    global _NC_CACHE
    if _NC_CACHE is None:
        _NC_CACHE = build_nc()
    return _NC_CACHE


def _tile_rows(a):
    """[D, N] -> [128, D//128, N] (partition-major SBUF layout)."""
    d, n = a.shape
    return np.ascontiguousarray(a.reshape(d // P, P, n).transpose(1, 0, 2))


def make_in_maps(x, Wq, Wk, Wv):
    M = (Wq.astype(np.float64) @ Wk.astype(np.float64).T).astype(np.float32)
    m16 = _tile_rows(M.astype(np.float16))
    wv16 = _tile_rows(Wv.astype(np.float16))
    wv16 = np.ascontiguousarray(
        wv16.reshape(P, DT, 2, 384).transpose(2, 0, 1, 3))
    mixed = SCHEME == "mixed"
    in_maps = []
    for c in range(N_CORES):
        xT = np.ascontiguousarray(x[c].T)  # [D, S] f32
        xc32 = np.ascontiguousarray(
            xT.reshape(DT, P, SC, 512).transpose(2, 1, 0, 3))
        im = {"x16": xc32.astype(np.float16), "m16": m16, "wv": wv16}
        if mixed:
            im["x32"] = xc32
        in_maps.append(im)
    return in_maps


def kernel(x, Wq, Wk, Wv):
    x = np.asarray(x, dtype=np.float32)
    Wq = np.asarray(Wq, dtype=np.float32)
    Wk = np.asarray(Wk, dtype=np.float32)
    Wv = np.asarray(Wv, dtype=np.float32)

    nc = _get_nc()
    in_maps = make_in_maps(x, Wq, Wk, Wv)
    res = run_bass_kernel_spmd(nc, in_maps, list(range(N_CORES)))
    out = np.stack(
        [res.results[c]["out"].transpose(1, 0, 2).reshape(S, H)
         for c in range(N_CORES)], axis=0)
    return out.astype(np.float32)


# revision 11
# speedup vs baseline: 1.0385x; 1.0200x over previous
"""Causal attention (B=8, S=2048, D=H=768) on 8 trn2 NeuronCores.

Data-parallel over batch: core c computes batch c entirely on-chip, no
collectives.  All matmuls contract over the partition dim.

Key algebraic move: scores = (x Wq)(x Wk)^T = x (Wq Wk^T) x^T, with
M = Wq Wk^T precomputed on host (768x768).  That folds the q AND k
projections into ONE on-device projection t = x M, and the scores'
k-side operand becomes raw x^T.

Precision scheme (HW-validated):
  - V = x Wv and t = x M run in single-pass fp16 (operand rounding only;
    PE accumulates exactly in fp32 PSUM).  t is stored as fp32 (float32r).
  - scores = t x^T runs as a SINGLE-PASS float32r matmul: fp32 operands
    in SBUF, ~2^-13.5 internal product truncation, 1.5 PE cycles/row --
    kills both the fp16 store-rounding of t and the k-side x rounding.
  - exp weights, transposes, and attn@V run in fp16; softmax stats fp32.

Per-core pipeline:
  phase 1b: V[s,h] = x16-blocks (stationary) x Wv16 (moving), fp16.
    The two head-critical DMAs (wv half 0, x16 chunk 0) are triggered
    from the Tensor queue, which is idle at t=0 (the Sync queue spends
    ~8us on semaphore init first).  Bulk loads (x32 chunks, m16) ride
    the SWDGE ring gated behind the first V copies.
  phase 1a: tT = M16 (stationary) x x16T (moving), fp16 -> f32r store.
    s-chunks run REVERSED (3,2,1,0) so the first phase-2 tiles (high qt)
    see their t chunk earliest.
  phase 2, per 128-row q-tile, qt descending 15..0: scores strip
    [q, k<=q] f32r; one DVE op per 512-chunk computes
    strip = causal_mask - psum (masked slots +1e10); a min-reduce gives
    -rowmax; exp on ScalarE (scale=-1, bias=-max, accum_out=rowsum)
    -> fp16; PE-transpose exp in batches of 4 blocks
    per PSUM tile with one copy each; out = sum_k expT x V (fp16);
    scale by 1/rowsum into an fp16 accumulator; one DMA per FOUR
    q-tiles ([128, 4*768] f16, partition-major DRAM layout) keeps the
    per-line descriptor overhead amortized and the end-of-kernel DMA
    drain short.

Host side: shards x over batch, pre-transposes/tiles, computes
M = Wq Wk^T in float64, replicates weights, gathers + de-tiles outputs.
"""

from contextlib import ExitStack

import numpy as np

import bass_rust
import concourse.mybir as mybir
import concourse.tile as tile
from concourse import bacc
from concourse.bass_utils import run_bass_kernel_spmd
from concourse.masks import make_causal_mask, make_identity

B, S, D, H = 8, 2048, 768, 768
N_CORES = 8
P = 128
DT = D // P    # 6 d-tiles
HT = H // P    # 6 h-tiles
ST = S // P    # 16 s-tiles
SC = S // 512  # 4 column-chunks

f32 = mybir.dt.float32
f32r = mybir.dt.float32r
f16 = mybir.dt.float16

# "mixed": scores in f32r (x32 shipped), t stored f32r.  rel err ~9e-3.
# "f16":   scores in fp16 (no x32), t stored f16.        rel err ~1.25e-2.
SCHEME = "mixed"


def _ceil_div(a, b):
    return (a + b - 1) // b


def build_nc(scheme=SCHEME):
    mixed = scheme == "mixed"
    nc = bacc.Bacc(None)

    # inputs ship pre-tiled from the host in exact SBUF layout
    # ([128 partitions, ...]) so every DMA line is fully contiguous
    x16_d = nc.declare_dram_parameter("x16", [SC, P, DT, 512], f16, isOutput=False)
    m16_d = nc.declare_dram_parameter("m16", [P, DT, D], f16, isOutput=False)
    wv_d = nc.declare_dram_parameter("wv", [2, P, DT, 384], f16, isOutput=False)
    if mixed:
        x32_d = nc.declare_dram_parameter(
            "x32", [SC, P, DT, 512], f32r, isOutput=False)
    # partition-major output: host de-tiles [P, ST, H] -> [S, H]
    out_d = nc.declare_dram_parameter("out", [P, ST, H], f16, isOutput=True)

    t_dt = f32r if mixed else f16

    with tile.TileContext(nc, pool_alloc_mode="queue") as tc, ExitStack() as ctx:
        persist = ctx.enter_context(tc.tile_pool(name="persist", bufs=1))
        t_s = persist.tile([P, HT, S], t_dt)     # 48KB/part (24 if f16)
        V = persist.tile([P, ST, H], f16)        # 24KB/part
        ident16 = persist.tile([P, P], f16)
        # zcm = [512 zeros | 128-col causal mask]; slice [640-w:640] puts
        # the mask on the last 128 of a w-wide diag chunk, zeros elsewhere
        zcm = persist.tile([P, 640], f32)
        if mixed:
            x32_s = persist.tile([P, SC, DT, 512], f32r)  # 48KB/part

        p1pool = tc.alloc_tile_pool(name="p1", bufs=1)
        x16_s = p1pool.tile([P, SC, DT, 512], f16)  # 24KB/part
        m16_s = p1pool.tile([P, DT, D], f16)
        wv_s = p1pool.tile([P, 2, DT, 384], f16)

        # head-critical loads first on the Sync queue: the first V matmul
        # group needs exactly these two transfers
        nc.sync.dma_start(out=wv_s[:, 0], in_=wv_d[0])
        nc.sync.dma_start(out=x16_s[:, 0], in_=x16_d[0])
        # near-term loads on the Sync queue
        nc.sync.dma_start(out=wv_s[:, 1], in_=wv_d[1])
        for sc in range(1, SC):
            nc.sync.dma_start(out=x16_s[:, sc], in_=x16_d[sc])

        make_identity(nc, ident16)
        nc.gpsimd.memset(zcm[:, 0:512], 0.0)
        make_causal_mask(nc, zcm[:, 512:640], mask_val=1e10)

        # ---- phase 1b: V = x16 (stationary) x Wv16 (moving) --------------
        with tc.tile_pool(name="p1b_ps", bufs=4, space="PSUM") as pp:
            bulk_anchor = None
            for sc in range(SC):
                if sc == 1:
                    # bulk loads (x32 24KB/part + m16) stream on the SWDGE
                    # ring once the head-critical transfers are done
                    assert bulk_anchor is not None
                    bulk = [(m16_s, m16_d[:, :, :])]
                    if mixed:
                        bulk += [(x32_s[:, c], x32_d[c]) for c in range(SC)]
                    for dst, src in bulk:
                        dma = nc.gpsimd.dma_start(out=dst, in_=src)
                        bass_rust.add_dep_helper(
                            dma.ins, bulk_anchor.ins, sync=True,
                            reason="bulk load waits for first V chunk")
                for hc in range(2):
                    for sti in range(4):
                        off = sti * P
                        ps = pp.tile([P, 384], f32, tag="psv", name="psv")
                        for dt_ in range(DT):
                            nc.tensor.matmul(
                                ps,
                                x16_s[:, sc, dt_, off:off + P],
                                wv_s[:, hc, dt_, :],
                                start=(dt_ == 0),
                                stop=(dt_ == DT - 1),
                            )
                        cp = nc.vector.tensor_copy(
                            V[:, sc * 4 + sti, hc * 384:(hc + 1) * 384], ps)
                        if sc == 0 and hc == 0 and sti == 3:
                            bulk_anchor = cp

        # ---- phase 1a: tT = M16 (stationary) x x16T (moving) -------------
        # reversed s-chunk order: phase 2 runs qt descending, so high-qt
        # tiles (which need the last t chunk for their stationary) unblock
        # right after the first chunk-group here
        with tc.tile_pool(name="p1a_ps", bufs=4, space="PSUM") as pp:
            for sc in range(SC - 1, -1, -1):
                for ht in range(HT):
                    ps = pp.tile([P, 512], f32, tag="ps", name="ps")
                    for dt_ in range(DT):
                        nc.tensor.matmul(
                            ps,
                            m16_s[:, dt_, ht * P:(ht + 1) * P],
                            x16_s[:, sc, dt_, :],
                            start=(dt_ == 0),
                            stop=(dt_ == DT - 1),
                        )
                    nc.scalar.copy(t_s[:, ht, sc * 512:(sc + 1) * 512], ps)
        if mixed:
            p1pool.release()
            xk_s = x32_s       # scores k-side operand
        else:
            xk_s = x16_s       # fp16 scores read x16 directly (persists)

        # ---- phase 2: attention ------------------------------------------
        with tc.tile_pool(name="p2_strip", bufs=3) as strip_pool, \
             tc.tile_pool(name="p2_exp", bufs=3) as exp_pool, \
             tc.tile_pool(name="p2_expT", bufs=3) as expT_pool, \
             tc.tile_pool(name="p2_stat", bufs=6) as stat_pool, \
             tc.tile_pool(name="p2_acc", bufs=2) as acc_pool, \
             tc.tile_pool(name="p2_ps_s", bufs=4, space="PSUM") as ps_s_pool, \
             tc.tile_pool(name="p2_ps_t", bufs=2, space="PSUM") as ps_t_pool, \
             tc.tile_pool(name="p2_ps_o", bufs=2, space="PSUM") as ps_o_pool:
            acc = None
            for qt in range(ST - 1, -1, -1):
                L = qt + 1
                cols = L * P
                nchn = _ceil_div(cols, 512)
                strip = strip_pool.tile([P, S], f32, tag="strip", name="strip")
                for nch in range(nchn):
                    w = min(512, cols - nch * 512)
                    # f32r needs moving >= 256 for full rate; pad short
                    # tails with throwaway columns
                    wp = max(w, 256) if mixed else w
                    ps = ps_s_pool.tile([P, 512], f32, tag="ps_s", name="ps_s")
                    for dt_ in range(DT):
                        nc.tensor.matmul(
                            ps[:, :wp],
                            t_s[:, dt_, qt * P:(qt + 1) * P],
                            xk_s[:, nch, dt_, 0:wp],
                            start=(dt_ == 0),
                            stop=(dt_ == DT - 1),
                        )
                    # strip = mask - scores (masked slots become +1e10,
                    # transparent to the min-reduce below)
                    in1 = zcm[:, 640 - w:640] if nch == nchn - 1 else zcm[:, 0:w]
                    nc.vector.tensor_sub(
                        strip[:, nch * 512:nch * 512 + w], in1, ps[:, :w])
                nmax = stat_pool.tile([P, 1], f32, tag="nmax", name="nmax")
                nc.vector.tensor_reduce(
                    nmax, strip[:, :cols],
                    axis=mybir.AxisListType.X, op=mybir.AluOpType.min,
                )
                rsum = stat_pool.tile([P, 1], f32, tag="rsum", name="rsum")
                exp16 = exp_pool.tile([P, S], f16, tag="exp16", name="exp16")
                # exp(in*-1 + (-max)) = exp(scores + mask - max)
                nc.scalar.activation(
                    exp16[:, :cols], strip[:, :cols],
                    mybir.ActivationFunctionType.Exp,
                    bias=nmax, scale=-1.0, accum_out=rsum,
                )
                rinv = stat_pool.tile([P, 1], f32, tag="rinv", name="rinv")
                nc.vector.reciprocal(rinv, rsum)
                expT = expT_pool.tile([P, ST, P], f16, tag="expT", name="expT")
                for j0 in range(0, L, 4):
                    jn = min(4, L - j0)
                    pst = ps_t_pool.tile([P, 512], f16, tag="ps_t", name="ps_t")
                    for i in range(jn):
                        nc.tensor.transpose(
                            pst[:, i * P:(i + 1) * P],
                            exp16[:, (j0 + i) * P:(j0 + i + 1) * P],
                            ident16)
                    nc.scalar.copy(expT[:, j0:j0 + jn, :], pst[:, :jn * P])
                if qt % 4 == 3:
                    acc = acc_pool.tile([P, 4, H], f16, tag="acc", name="acc")
                for hc in range(2):
                    pso = ps_o_pool.tile([P, 384], f32, tag="ps_o", name="ps_o")
                    for j in range(L):
                        nc.tensor.matmul(
                            pso,
                            expT[:, j, :],
                            V[:, j, hc * 384:(hc + 1) * 384],
                            start=(j == 0),
                            stop=(j == L - 1),
                        )
                    nc.vector.tensor_scalar_mul(
                        acc[:, qt % 4, hc * 384:(hc + 1) * 384], pso, rinv)
                if qt % 4 == 0:
                    nc.sync.dma_start(
                        out=out_d[:, qt:qt + 4, :], in_=acc)

    nc.finalize()
    return nc


_NC_CACHE = None


def _get_nc():
    global _NC_CACHE
    if _NC_CACHE is None:
        _NC_CACHE = build_nc()
    return _NC_CACHE


def _tile_rows(a):
    """[D, N] -> [128, D//128, N] (partition-major SBUF layout)."""
    d, n = a.shape
    return np.ascontiguousarray(a.reshape(d // P, P, n).transpose(1, 0, 2))


def make_in_maps(x, Wq, Wk, Wv):
    M = (Wq.astype(np.float64) @ Wk.astype(np.float64).T).astype(np.float32)
    m16 = _tile_rows(M.astype(np.float16))
    wv16 = _tile_rows(Wv.astype(np.float16))
    wv16 = np.ascontiguousarray(
        wv16.reshape(P, DT, 2, 384).transpose(2, 0, 1, 3))
    mixed = SCHEME == "mixed"
    in_maps = []
    for c in range(N_CORES):
        xT = np.ascontiguousarray(x[c].T)  # [D, S] f32
        xc32 = np.ascontiguousarray(
            xT.reshape(DT, P, SC, 512).transpose(2, 1, 0, 3))
        im = {"x16": xc32.astype(np.float16), "m16": m16, "wv": wv16}
        if mixed:
            im["x32"] = xc32
        in_maps.append(im)
    return in_maps


def kernel(x, Wq, Wk, Wv):
    x = np.asarray(x, dtype=np.float32)
    Wq = np.asarray(Wq, dtype=np.float32)
    Wk = np.asarray(Wk, dtype=np.float32)
    Wv = np.asarray(Wv, dtype=np.float32)

    nc = _get_nc()
    in_maps = make_in_maps(x, Wq, Wk, Wv)
    res = run_bass_kernel_spmd(nc, in_maps, list(range(N_CORES)))
    out = np.stack(
        [res.results[c]["out"].transpose(1, 0, 2).reshape(S, H)
         for c in range(N_CORES)], axis=0)
    return out.astype(np.float32)


# revision 13
# speedup vs baseline: 1.0614x; 1.0220x over previous
"""Causal attention (B=8, S=2048, D=H=768) on 8 trn2 NeuronCores.

Data-parallel over batch: core c computes batch c entirely on-chip, no
collectives.  All matmuls contract over the partition dim.

Key algebraic move: scores = (x Wq)(x Wk)^T = x (Wq Wk^T) x^T, with
M = Wq Wk^T precomputed on host (768x768).  That folds the q AND k
projections into ONE on-device projection t = x M, and the scores'
k-side operand becomes raw x^T.

Precision scheme (HW-validated):
  - V = x Wv and t = x M run in single-pass fp16 (operand rounding only;
    PE accumulates exactly in fp32 PSUM).  t is stored as fp32 (float32r).
  - scores = t x^T runs as a SINGLE-PASS float32r matmul: fp32 operands
    in SBUF, ~2^-13.5 internal product truncation, 1.5 PE cycles/row --
    kills both the fp16 store-rounding of t and the k-side x rounding.
  - exp weights, transposes, and attn@V run in fp16; softmax stats fp32.

Per-core pipeline:
  phase 1b: V[s,h] = x16-blocks (stationary) x Wv16 (moving), fp16.
    The two head-critical DMAs (wv half 0, x16 chunk 0) are triggered
    from the Tensor queue, which is idle at t=0 (the Sync queue spends
    ~8us on semaphore init first).  Bulk loads (x32 chunks, m16) ride
    the SWDGE ring gated behind the first V copies.
  phase 1a: tT = M16 (stationary) x x16T (moving), fp16 -> f32r store.
    s-chunks run REVERSED (3,2,1,0) so the first phase-2 tiles (high qt)
    see their t chunk earliest.
  phase 2, per 128-row q-tile, qt descending 15..0: scores strip
    [q, k<=q] f32r; one DVE op per 512-chunk computes
    strip = causal_mask - psum (masked slots +1e10); a min-reduce gives
    -rowmax; exp on ScalarE (scale=-1, bias=-max, accum_out=rowsum)
    -> fp16; PE-transpose exp in batches of 4 blocks
    per PSUM tile with one copy each; out = sum_k expT x V (fp16);
    scale by 1/rowsum into an fp16 accumulator; one DMA per FOUR
    q-tiles ([128, 4*768] f16, partition-major DRAM layout) keeps the
    per-line descriptor overhead amortized and the end-of-kernel DMA
    drain short.

Host side: shards x over batch, pre-transposes/tiles, computes
M = Wq Wk^T in float64, replicates weights, gathers + de-tiles outputs.
"""

from contextlib import ExitStack

import numpy as np

import bass_rust
import concourse.mybir as mybir
import concourse.tile as tile
from concourse import bacc
from concourse.bass_utils import run_bass_kernel_spmd
from concourse.masks import make_causal_mask, make_identity

B, S, D, H = 8, 2048, 768, 768
N_CORES = 8
P = 128
DT = D // P    # 6 d-tiles
HT = H // P    # 6 h-tiles
ST = S // P    # 16 s-tiles
SC = S // 512  # 4 column-chunks

f32 = mybir.dt.float32
f32r = mybir.dt.float32r
f16 = mybir.dt.float16

# "mixed": scores in f32r (x32 shipped), t stored f32r.  rel err ~9e-3.
# "f16":   scores in fp16 (no x32), t stored f16.        rel err ~1.25e-2.
SCHEME = "f16"


def _ceil_div(a, b):
    return (a + b - 1) // b


def build_nc(scheme=SCHEME):
    mixed = scheme == "mixed"
    nc = bacc.Bacc(None)

    # inputs ship pre-tiled from the host in exact SBUF layout
    # ([128 partitions, ...]) so every DMA line is fully contiguous
    x16_d = nc.declare_dram_parameter("x16", [SC, P, DT, 512], f16, isOutput=False)
    m16_d = nc.declare_dram_parameter("m16", [P, DT, D], f16, isOutput=False)
    wv_d = nc.declare_dram_parameter("wv", [2, P, DT, 384], f16, isOutput=False)
    if mixed:
        x32_d = nc.declare_dram_parameter(
            "x32", [SC, P, DT, 512], f32r, isOutput=False)
    # partition-major output: host de-tiles [P, ST, H] -> [S, H]
    out_d = nc.declare_dram_parameter("out", [P, ST, H], f16, isOutput=True)

    t_dt = f32r if mixed else f16

    with tile.TileContext(nc, pool_alloc_mode="queue") as tc, ExitStack() as ctx:
        persist = ctx.enter_context(tc.tile_pool(name="persist", bufs=1))
        t_s = persist.tile([P, HT, S], t_dt)     # 48KB/part (24 if f16)
        V = persist.tile([P, ST, H], f16)        # 24KB/part
        ident16 = persist.tile([P, P], f16)
        # zcm = [512 zeros | 128-col causal mask]; slice [640-w:640] puts
        # the mask on the last 128 of a w-wide diag chunk, zeros elsewhere
        zcm = persist.tile([P, 640], f32)
        if mixed:
            x32_s = persist.tile([P, SC, DT, 512], f32r)  # 48KB/part

        p1pool = tc.alloc_tile_pool(name="p1", bufs=1)
        if mixed:
            x16_s = p1pool.tile([P, SC, DT, 512], f16)  # 24KB/part
        else:
            x16_s = persist.tile([P, SC, DT, 512], f16)  # k-side operand
        m16_s = p1pool.tile([P, DT, D], f16)
        wv_s = p1pool.tile([P, 2, DT, 384], f16)

        # head-critical loads first on the Sync queue: the first V matmul
        # group needs exactly these two transfers
        nc.sync.dma_start(out=wv_s[:, 0], in_=wv_d[0])
        nc.sync.dma_start(out=x16_s[:, 0], in_=x16_d[0])
        # near-term loads on the Sync queue
        nc.sync.dma_start(out=wv_s[:, 1], in_=wv_d[1])
        for sc in range(1, SC):
            nc.sync.dma_start(out=x16_s[:, sc], in_=x16_d[sc])

        make_identity(nc, ident16)
        nc.gpsimd.memset(zcm[:, 0:512], 0.0)
        make_causal_mask(nc, zcm[:, 512:640], mask_val=1e10)

        # ---- phase 1b: V = x16 (stationary) x Wv16 (moving) --------------
        with tc.tile_pool(name="p1b_ps", bufs=4, space="PSUM") as pp:
            bulk_anchor = None
            for sc in range(SC):
                if sc == 1:
                    # bulk loads (x32 24KB/part + m16) stream on the SWDGE
                    # ring once the head-critical transfers are done
                    assert bulk_anchor is not None
                    bulk = [(m16_s, m16_d[:, :, :])]
                    if mixed:
                        bulk += [(x32_s[:, c], x32_d[c]) for c in range(SC)]
                    for dst, src in bulk:
                        dma = nc.gpsimd.dma_start(out=dst, in_=src)
                        bass_rust.add_dep_helper(
                            dma.ins, bulk_anchor.ins, sync=True,
                            reason="bulk load waits for first V chunk")
                for hc in range(2):
                    for sti in range(4):
                        off = sti * P
                        ps = pp.tile([P, 384], f32, tag="psv", name="psv")
                        for dt_ in range(DT):
                            nc.tensor.matmul(
                                ps,
                                x16_s[:, sc, dt_, off:off + P],
                                wv_s[:, hc, dt_, :],
                                start=(dt_ == 0),
                                stop=(dt_ == DT - 1),
                            )
                        cp = nc.vector.tensor_copy(
                            V[:, sc * 4 + sti, hc * 384:(hc + 1) * 384], ps)
                        if sc == 0 and hc == 0 and sti == 3:
                            bulk_anchor = cp

        # ---- phase 1a: tT = M16 (stationary) x x16T (moving) -------------
        # reversed s-chunk order: phase 2 runs qt descending, so high-qt
        # tiles (which need the last t chunk for their stationary) unblock
        # right after the first chunk-group here
        with tc.tile_pool(name="p1a_ps", bufs=4, space="PSUM") as pp:
            for sc in range(SC - 1, -1, -1):
                for ht in range(HT):
                    ps = pp.tile([P, 512], f32, tag="ps", name="ps")
                    for dt_ in range(DT):
                        nc.tensor.matmul(
                            ps,
                            m16_s[:, dt_, ht * P:(ht + 1) * P],
                            x16_s[:, sc, dt_, :],
                            start=(dt_ == 0),
                            stop=(dt_ == DT - 1),
                        )
                    nc.scalar.copy(t_s[:, ht, sc * 512:(sc + 1) * 512], ps)
        p1pool.release()
        xk_s = x32_s if mixed else x16_s  # scores k-side operand

        # ---- phase 2: attention ------------------------------------------
        with tc.tile_pool(name="p2_strip", bufs=3) as strip_pool, \
             tc.tile_pool(name="p2_exp", bufs=3) as exp_pool, \
             tc.tile_pool(name="p2_expT", bufs=3) as expT_pool, \
             tc.tile_pool(name="p2_stat", bufs=6) as stat_pool, \
             tc.tile_pool(name="p2_acc", bufs=2) as acc_pool, \
             tc.tile_pool(name="p2_ps_s", bufs=4, space="PSUM") as ps_s_pool, \
             tc.tile_pool(name="p2_ps_t", bufs=2, space="PSUM") as ps_t_pool, \
             tc.tile_pool(name="p2_ps_o", bufs=2, space="PSUM") as ps_o_pool:
            acc = None
            for qt in range(ST - 1, -1, -1):
                L = qt + 1
                cols = L * P
                nchn = _ceil_div(cols, 512)
                strip = strip_pool.tile([P, S], f32, tag="strip", name="strip")
                for nch in range(nchn):
                    w = min(512, cols - nch * 512)
                    # f32r needs moving >= 256 for full rate; pad short
                    # tails with throwaway columns
                    wp = max(w, 256) if mixed else w
                    ps = ps_s_pool.tile([P, 512], f32, tag="ps_s", name="ps_s")
                    for dt_ in range(DT):
                        nc.tensor.matmul(
                            ps[:, :wp],
                            t_s[:, dt_, qt * P:(qt + 1) * P],
                            xk_s[:, nch, dt_, 0:wp],
                            start=(dt_ == 0),
                            stop=(dt_ == DT - 1),
                        )
                    # strip = mask - scores (masked slots become +1e10,
                    # transparent to the min-reduce below)
                    in1 = zcm[:, 640 - w:640] if nch == nchn - 1 else zcm[:, 0:w]
                    nc.vector.tensor_sub(
                        strip[:, nch * 512:nch * 512 + w], in1, ps[:, :w])
                nmax = stat_pool.tile([P, 1], f32, tag="nmax", name="nmax")
                nc.vector.tensor_reduce(
                    nmax, strip[:, :cols],
                    axis=mybir.AxisListType.X, op=mybir.AluOpType.min,
                )
                rsum = stat_pool.tile([P, 1], f32, tag="rsum", name="rsum")
                exp16 = exp_pool.tile([P, S], f16, tag="exp16", name="exp16")
                # exp(in*-1 + (-max)) = exp(scores + mask - max)
                nc.scalar.activation(
                    exp16[:, :cols], strip[:, :cols],
                    mybir.ActivationFunctionType.Exp,
                    bias=nmax, scale=-1.0, accum_out=rsum,
                )
                rinv = stat_pool.tile([P, 1], f32, tag="rinv", name="rinv")
                nc.vector.reciprocal(rinv, rsum)
                expT = expT_pool.tile([P, ST, P], f16, tag="expT", name="expT")
                for j0 in range(0, L, 4):
                    jn = min(4, L - j0)
                    pst = ps_t_pool.tile([P, 512], f16, tag="ps_t", name="ps_t")
                    for i in range(jn):
                        nc.tensor.transpose(
                            pst[:, i * P:(i + 1) * P],
                            exp16[:, (j0 + i) * P:(j0 + i + 1) * P],
                            ident16)
                    nc.scalar.copy(expT[:, j0:j0 + jn, :], pst[:, :jn * P])
                if qt % 4 == 3:
                    acc = acc_pool.tile([P, 4, H], f16, tag="acc", name="acc")
                for hc in range(2):
                    pso = ps_o_pool.tile([P, 384], f32, tag="ps_o", name="ps_o")
                    for j in range(L):
                        nc.tensor.matmul(
                            pso,
                            expT[:, j, :],
                            V[:, j, hc * 384:(hc + 1) * 384],
                            start=(j == 0),
                            stop=(j == L - 1),
                        )
                    nc.vector.tensor_scalar_mul(
                        acc[:, qt % 4, hc * 384:(hc + 1) * 384], pso, rinv)
                if qt % 4 == 0:
                    nc.sync.dma_start(
                        out=out_d[:, qt:qt + 4, :], in_=acc)

    nc.finalize()
    return nc


_NC_CACHE = None


def _get_nc():
    global _NC_CACHE
    if _NC_CACHE is None:
        _NC_CACHE = build_nc()
    return _NC_CACHE


def _tile_rows(a):
    """[D, N] -> [128, D//128, N] (partition-major SBUF layout)."""
    d, n = a.shape
    return np.ascontiguousarray(a.reshape(d // P, P, n).transpose(1, 0, 2))


def make_in_maps(x, Wq, Wk, Wv):
    M = (Wq.astype(np.float64) @ Wk.astype(np.float64).T).astype(np.float32)
    m16 = _tile_rows(M.astype(np.float16))
    wv16 = _tile_rows(Wv.astype(np.float16))
    wv16 = np.ascontiguousarray(
        wv16.reshape(P, DT, 2, 384).transpose(2, 0, 1, 3))
    mixed = SCHEME == "mixed"
    in_maps = []
    for c in range(N_CORES):
        xT = np.ascontiguousarray(x[c].T)  # [D, S] f32
        xc32 = np.ascontiguousarray(
            xT.reshape(DT, P, SC, 512).transpose(2, 1, 0, 3))
        im = {"x16": xc32.astype(np.float16), "m16": m16, "wv": wv16}
        if mixed:
            im["x32"] = xc32
        in_maps.append(im)
    return in_maps


def kernel(x, Wq, Wk, Wv):
    x = np.asarray(x, dtype=np.float32)
    Wq = np.asarray(Wq, dtype=np.float32)
    Wk = np.asarray(Wk, dtype=np.float32)
    Wv = np.asarray(Wv, dtype=np.float32)

    nc = _get_nc()
    in_maps = make_in_maps(x, Wq, Wk, Wv)
    res = run_bass_kernel_spmd(nc, in_maps, list(range(N_CORES)))
    out = np.stack(
        [res.results[c]["out"].transpose(1, 0, 2).reshape(S, H)
         for c in range(N_CORES)], axis=0)
    return out.astype(np.float32)


# revision 15
# speedup vs baseline: 1.0734x; 1.0114x over previous
"""Causal attention (B=8, S=2048, D=H=768) on 8 trn2 NeuronCores.

Data-parallel over batch: core c computes batch c entirely on-chip, no
collectives.  All matmuls contract over the partition dim.

Key algebraic move: scores = (x Wq)(x Wk)^T = x (Wq Wk^T) x^T, with
M = Wq Wk^T precomputed on host (768x768).  That folds the q AND k
projections into ONE on-device projection t = x M, and the scores'
k-side operand becomes raw x^T.

Precision scheme (HW-validated):
  - V = x Wv and t = x M run in single-pass fp16 (operand rounding only;
    PE accumulates exactly in fp32 PSUM).  t is stored as fp32 (float32r).
  - scores = t x^T runs as a SINGLE-PASS float32r matmul: fp32 operands
    in SBUF, ~2^-13.5 internal product truncation, 1.5 PE cycles/row --
    kills both the fp16 store-rounding of t and the k-side x rounding.
  - exp weights, transposes, and attn@V run in fp16; softmax stats fp32.

Per-core pipeline:
  phase 1b: V[s,h] = x16-blocks (stationary) x Wv16 (moving), fp16.
    The two head-critical DMAs (wv half 0, x16 chunk 0) are triggered
    from the Tensor queue, which is idle at t=0 (the Sync queue spends
    ~8us on semaphore init first).  Bulk loads (x32 chunks, m16) ride
    the SWDGE ring gated behind the first V copies.
  phase 1a: tT = M16 (stationary) x x16T (moving), fp16 -> f32r store.
    s-chunks run REVERSED (3,2,1,0) so the first phase-2 tiles (high qt)
    see their t chunk earliest.
  phase 2, per 128-row q-tile, qt descending 15..0: scores strip
    [q, k<=q] f32r; one DVE op per 512-chunk computes
    strip = causal_mask - psum (masked slots +1e10); a min-reduce gives
    -rowmax; exp on ScalarE (scale=-1, bias=-max, accum_out=rowsum)
    -> fp16; PE-transpose exp in batches of 4 blocks
    per PSUM tile with one copy each; out = sum_k expT x V (fp16);
    scale by 1/rowsum into an fp16 accumulator; one DMA per FOUR
    q-tiles ([128, 4*768] f16, partition-major DRAM layout) keeps the
    per-line descriptor overhead amortized and the end-of-kernel DMA
    drain short.

Host side: shards x over batch, pre-transposes/tiles, computes
M = Wq Wk^T in float64, replicates weights, gathers + de-tiles outputs.
"""

from contextlib import ExitStack

import numpy as np

import bass_rust
import concourse.mybir as mybir
import concourse.tile as tile
from concourse import bacc
from concourse.bass_utils import run_bass_kernel_spmd
from concourse.masks import make_causal_mask, make_identity

B, S, D, H = 8, 2048, 768, 768
N_CORES = 8
P = 128
DT = D // P    # 6 d-tiles
HT = H // P    # 6 h-tiles
ST = S // P    # 16 s-tiles
SC = S // 512  # 4 column-chunks

f32 = mybir.dt.float32
f32r = mybir.dt.float32r
f16 = mybir.dt.float16

# "mixed": scores in f32r (x32 shipped), t stored f32r.  rel err ~9e-3.
# "f16":   scores in fp16 (no x32), t stored f16.        rel err ~1.25e-2.
SCHEME = "f16"


def _ceil_div(a, b):
    return (a + b - 1) // b


def build_nc(scheme=SCHEME):
    mixed = scheme == "mixed"
    nc = bacc.Bacc(None)

    # inputs ship pre-tiled from the host in exact SBUF layout
    # ([128 partitions, ...]) so every DMA line is fully contiguous
    # head = [wv half 0 | x16 chunk 0] combined: ONE 128-line DMA gates
    # the first V matmul group instead of two
    head_d = nc.declare_dram_parameter("head", [P, DT, 896], f16, isOutput=False)
    x16_d = nc.declare_dram_parameter(
        "x16", [SC - 1, P, DT, 512], f16, isOutput=False)
    m16_d = nc.declare_dram_parameter("m16", [P, DT, D], f16, isOutput=False)
    wv_d = nc.declare_dram_parameter("wv", [2, P, DT, 384], f16, isOutput=False)
    if mixed:
        x32_d = nc.declare_dram_parameter(
            "x32", [SC, P, DT, 512], f32r, isOutput=False)
    # partition-major output: host de-tiles [P, ST, H] -> [S, H]
    out_d = nc.declare_dram_parameter("out", [P, ST, H], f16, isOutput=True)

    t_dt = f32r if mixed else f16

    with tile.TileContext(nc, pool_alloc_mode="queue") as tc, ExitStack() as ctx:
        persist = ctx.enter_context(tc.tile_pool(name="persist", bufs=1))
        t_s = persist.tile([P, HT, S], t_dt)     # 48KB/part (24 if f16)
        V = persist.tile([P, ST, H], f16)        # 24KB/part
        ident16 = persist.tile([P, P], f16)
        # zcm = [512 zeros | 128-col causal mask]; slice [640-w:640] puts
        # the mask on the last 128 of a w-wide diag chunk, zeros elsewhere
        zcm = persist.tile([P, 640], f32)
        if mixed:
            x32_s = persist.tile([P, SC, DT, 512], f32r)  # 48KB/part

        p1pool = tc.alloc_tile_pool(name="p1", bufs=1)
        hpool = p1pool if mixed else persist
        hd_s = hpool.tile([P, DT, 896], f16)        # wv0 | x16 chunk 0
        x16t = hpool.tile([P, SC - 1, DT, 512], f16)  # x16 chunks 1..3
        m16_s = p1pool.tile([P, DT, D], f16)
        wv1_s = p1pool.tile([P, DT, 384], f16)

        def x16_s(sc, dt_, a, b):
            if sc == 0:
                return hd_s[:, dt_, 384 + a:384 + b]
            return x16t[:, sc - 1, dt_, a:b]

        # head-critical combined load first on the Sync queue: the first
        # V matmul group needs exactly this one transfer
        nc.sync.dma_start(out=hd_s, in_=head_d[:, :, :])
        # near-term loads on the Sync queue
        nc.sync.dma_start(out=wv1_s, in_=wv_d[1])
        for sc in range(1, SC):
            nc.sync.dma_start(out=x16t[:, sc - 1], in_=x16_d[sc - 1])

        # pre-ramp the PE clock (HAM releases the throttle after ~4us of
        # sustained activity) with junk matmuls while the head DMA lands
        with tc.tile_pool(name="warm", bufs=1) as warm_pool, \
             tc.tile_pool(name="warm_ps", bufs=1, space="PSUM") as warm_pp:
            junk = warm_pool.tile([P, 512], f16)
            nc.vector.memset(junk, 0.125)
            wps = warm_pp.tile([P, 512], f32, tag="wps", name="wps")
            for i in range(14):
                nc.tensor.matmul(
                    wps, junk[:, 0:P], junk,
                    start=(i == 0), stop=(i == 13))

        make_identity(nc, ident16)
        nc.gpsimd.memset(zcm[:, 0:512], 0.0)
        make_causal_mask(nc, zcm[:, 512:640], mask_val=1e10)

        # ---- phase 1b: V = x16 (stationary) x Wv16 (moving) --------------
        with tc.tile_pool(name="p1b_ps", bufs=4, space="PSUM") as pp:
            bulk_anchor = None
            for sc in range(SC):
                if sc == 1:
                    # bulk loads (x32 24KB/part + m16) stream on the SWDGE
                    # ring once the head-critical transfers are done
                    assert bulk_anchor is not None
                    bulk = [(m16_s, m16_d[:, :, :])]
                    if mixed:
                        bulk += [(x32_s[:, c], x32_d[c]) for c in range(SC)]
                    for dst, src in bulk:
                        dma = nc.gpsimd.dma_start(out=dst, in_=src)
                        bass_rust.add_dep_helper(
                            dma.ins, bulk_anchor.ins, sync=True,
                            reason="bulk load waits for first V chunk")
                for hc in range(2):
                    for sti in range(4):
                        off = sti * P
                        ps = pp.tile([P, 384], f32, tag="psv", name="psv")
                        for dt_ in range(DT):
                            nc.tensor.matmul(
                                ps,
                                x16_s(sc, dt_, off, off + P),
                                hd_s[:, dt_, 0:384] if hc == 0
                                else wv1_s[:, dt_, :],
                                start=(dt_ == 0),
                                stop=(dt_ == DT - 1),
                            )
                        cp = nc.vector.tensor_copy(
                            V[:, sc * 4 + sti, hc * 384:(hc + 1) * 384], ps)
                        if sc == 0 and hc == 0 and sti == 3:
                            bulk_anchor = cp

        # ---- phase 1a: tT = M16 (stationary) x x16T (moving) -------------
        # reversed s-chunk order: phase 2 runs qt descending, so high-qt
        # tiles (which need the last t chunk for their stationary) unblock
        # right after the first chunk-group here
        with tc.tile_pool(name="p1a_ps", bufs=4, space="PSUM") as pp:
            for sc in range(SC - 1, -1, -1):
                for ht in range(HT):
                    ps = pp.tile([P, 512], f32, tag="ps", name="ps")
                    for dt_ in range(DT):
                        nc.tensor.matmul(
                            ps,
                            m16_s[:, dt_, ht * P:(ht + 1) * P],
                            x16_s(sc, dt_, 0, 512),
                            start=(dt_ == 0),
                            stop=(dt_ == DT - 1),
                        )
                    nc.scalar.copy(t_s[:, ht, sc * 512:(sc + 1) * 512], ps)
        p1pool.release()
        if mixed:
            def xk_s(nch, dt_, a, b):
                return x32_s[:, nch, dt_, a:b]
        else:
            xk_s = x16_s  # scores k-side reads x16 chunks directly

        # ---- phase 2: attention ------------------------------------------
        with tc.tile_pool(name="p2_strip", bufs=4) as strip_pool, \
             tc.tile_pool(name="p2_exp", bufs=4) as exp_pool, \
             tc.tile_pool(name="p2_expT", bufs=4) as expT_pool, \
             tc.tile_pool(name="p2_stat", bufs=8) as stat_pool, \
             tc.tile_pool(name="p2_acc", bufs=2) as acc_pool, \
             tc.tile_pool(name="p2_ps_s", bufs=4, space="PSUM") as ps_s_pool, \
             tc.tile_pool(name="p2_ps_t", bufs=2, space="PSUM") as ps_t_pool, \
             tc.tile_pool(name="p2_ps_o", bufs=2, space="PSUM") as ps_o_pool:
            acc = None
            for qt in range(ST - 1, -1, -1):
                L = qt + 1
                cols = L * P
                nchn = _ceil_div(cols, 512)
                strip = strip_pool.tile([P, S], f32, tag="strip", name="strip")
                for nch in range(nchn):
                    w = min(512, cols - nch * 512)
                    # f32r needs moving >= 256 for full rate; pad short
                    # tails with throwaway columns
                    wp = max(w, 256) if mixed else w
                    ps = ps_s_pool.tile([P, 512], f32, tag="ps_s", name="ps_s")
                    for dt_ in range(DT):
                        nc.tensor.matmul(
                            ps[:, :wp],
                            t_s[:, dt_, qt * P:(qt + 1) * P],
                            xk_s(nch, dt_, 0, wp),
                            start=(dt_ == 0),
                            stop=(dt_ == DT - 1),
                        )
                    # strip = mask - scores (masked slots become +1e10,
                    # transparent to the min-reduce below)
                    in1 = zcm[:, 640 - w:640] if nch == nchn - 1 else zcm[:, 0:w]
                    nc.vector.tensor_sub(
                        strip[:, nch * 512:nch * 512 + w], in1, ps[:, :w])
                nmax = stat_pool.tile([P, 1], f32, tag="nmax", name="nmax")
                nc.vector.tensor_reduce(
                    nmax, strip[:, :cols],
                    axis=mybir.AxisListType.X, op=mybir.AluOpType.min,
                )
                rsum = stat_pool.tile([P, 1], f32, tag="rsum", name="rsum")
                exp16 = exp_pool.tile([P, S], f16, tag="exp16", name="exp16")
                # exp(in*-1 + (-max)) = exp(scores + mask - max)
                nc.scalar.activation(
                    exp16[:, :cols], strip[:, :cols],
                    mybir.ActivationFunctionType.Exp,
                    bias=nmax, scale=-1.0, accum_out=rsum,
                )
                rinv = stat_pool.tile([P, 1], f32, tag="rinv", name="rinv")
                nc.vector.reciprocal(rinv, rsum)
                expT = expT_pool.tile([P, ST, P], f16, tag="expT", name="expT")
                for j0 in range(0, L, 4):
                    jn = min(4, L - j0)
                    pst = ps_t_pool.tile([P, 512], f16, tag="ps_t", name="ps_t")
                    for i in range(jn):
                        nc.tensor.transpose(
                            pst[:, i * P:(i + 1) * P],
                            exp16[:, (j0 + i) * P:(j0 + i + 1) * P],
                            ident16)
                    nc.scalar.copy(expT[:, j0:j0 + jn, :], pst[:, :jn * P])
                if qt % 4 == 3:
                    acc = acc_pool.tile([P, 4, H], f16, tag="acc", name="acc")
                for hc in range(2):
                    pso = ps_o_pool.tile([P, 384], f32, tag="ps_o", name="ps_o")
                    for j in range(L):
                        nc.tensor.matmul(
                            pso,
                            expT[:, j, :],
                            V[:, j, hc * 384:(hc + 1) * 384],
                            start=(j == 0),
                            stop=(j == L - 1),
                        )
                    nc.vector.tensor_scalar_mul(
                        acc[:, qt % 4, hc * 384:(hc + 1) * 384], pso, rinv)
                if qt % 4 == 0:
                    nc.sync.dma_start(
                        out=out_d[:, qt:qt + 4, :], in_=acc)

    nc.finalize()
    return nc


_NC_CACHE = None


def _get_nc():
    global _NC_CACHE
    if _NC_CACHE is None:
        _NC_CACHE = build_nc()
    return _NC_CACHE


def _tile_rows(a):
    """[D, N] -> [128, D//128, N] (partition-major SBUF layout)."""
    d, n = a.shape
    return np.ascontiguousarray(a.reshape(d // P, P, n).transpose(1, 0, 2))


def make_in_maps(x, Wq, Wk, Wv):
    M = (Wq.astype(np.float64) @ Wk.astype(np.float64).T).astype(np.float32)
    m16 = _tile_rows(M.astype(np.float16))
    wv16 = _tile_rows(Wv.astype(np.float16))
    wv16 = np.ascontiguousarray(
        wv16.reshape(P, DT, 2, 384).transpose(2, 0, 1, 3))
    mixed = SCHEME == "mixed"
    in_maps = []
    for c in range(N_CORES):
        xT = np.ascontiguousarray(x[c].T)  # [D, S] f32
        xc32 = np.ascontiguousarray(
            xT.reshape(DT, P, SC, 512).transpose(2, 1, 0, 3))
        xc16 = xc32.astype(np.float16)
        head = np.ascontiguousarray(
            np.concatenate([wv16[0], xc16[0]], axis=-1))
        im = {"head": head, "x16": np.ascontiguousarray(xc16[1:]),
              "m16": m16, "wv": wv16}
        if mixed:
            im["x32"] = xc32
        in_maps.append(im)
    return in_maps


def kernel(x, Wq, Wk, Wv):
    x = np.asarray(x, dtype=np.float32)
    Wq = np.asarray(Wq, dtype=np.float32)
    Wk = np.asarray(Wk, dtype=np.float32)
    Wv = np.asarray(Wv, dtype=np.float32)

    nc = _get_nc()
    in_maps = make_in_maps(x, Wq, Wk, Wv)
    res = run_bass_kernel_spmd(nc, in_maps, list(range(N_CORES)))
    out = np.stack(
        [res.results[c]["out"].transpose(1, 0, 2).reshape(S, H)
         for c in range(N_CORES)], axis=0)
    return out.astype(np.float32)


# revision 17
# speedup vs baseline: 1.0798x; 1.0059x over previous
"""Causal attention (B=8, S=2048, D=H=768) on 8 trn2 NeuronCores.

Data-parallel over batch: core c computes batch c entirely on-chip, no
collectives.  All matmuls contract over the partition dim.

Key algebraic move: scores = (x Wq)(x Wk)^T = x (Wq Wk^T) x^T, with
M = Wq Wk^T precomputed on host (768x768).  That folds the q AND k
projections into ONE on-device projection t = x M, and the scores'
k-side operand becomes raw x^T.

Precision scheme (HW-validated):
  - V = x Wv and t = x M run in single-pass fp16 (operand rounding only;
    PE accumulates exactly in fp32 PSUM).  t is stored as fp32 (float32r).
  - scores = t x^T runs as a SINGLE-PASS float32r matmul: fp32 operands
    in SBUF, ~2^-13.5 internal product truncation, 1.5 PE cycles/row --
    kills both the fp16 store-rounding of t and the k-side x rounding.
  - exp weights, transposes, and attn@V run in fp16; softmax stats fp32.

Per-core pipeline:
  phase 1b: V[s,h] = x16-blocks (stationary) x Wv16 (moving), fp16.
    The two head-critical DMAs (wv half 0, x16 chunk 0) are triggered
    from the Tensor queue, which is idle at t=0 (the Sync queue spends
    ~8us on semaphore init first).  Bulk loads (x32 chunks, m16) ride
    the SWDGE ring gated behind the first V copies.
  phase 1a: tT = M16 (stationary) x x16T (moving), fp16 -> f32r store.
    s-chunks run REVERSED (3,2,1,0) so the first phase-2 tiles (high qt)
    see their t chunk earliest.
  phase 2, per 128-row q-tile, qt descending 15..0: scores strip
    [q, k<=q] f32r; one DVE op per 512-chunk computes
    strip = causal_mask - psum (masked slots +1e10); a min-reduce gives
    -rowmax; exp on ScalarE (scale=-1, bias=-max, accum_out=rowsum)
    -> fp16; PE-transpose exp in batches of 4 blocks
    per PSUM tile with one copy each; out = sum_k expT x V (fp16);
    scale by 1/rowsum into an fp16 accumulator; one DMA per FOUR
    q-tiles ([128, 4*768] f16, partition-major DRAM layout) keeps the
    per-line descriptor overhead amortized and the end-of-kernel DMA
    drain short.

Host side: shards x over batch, pre-transposes/tiles, computes
M = Wq Wk^T in float64, replicates weights, gathers + de-tiles outputs.
"""

from contextlib import ExitStack

import numpy as np

import bass_rust
import concourse.mybir as mybir
import concourse.tile as tile
from concourse import bacc
from concourse.bass_utils import run_bass_kernel_spmd
from concourse.masks import make_causal_mask, make_identity

B, S, D, H = 8, 2048, 768, 768
N_CORES = 8
P = 128
DT = D // P    # 6 d-tiles
HT = H // P    # 6 h-tiles
ST = S // P    # 16 s-tiles
SC = S // 512  # 4 column-chunks

f32 = mybir.dt.float32
f32r = mybir.dt.float32r
f16 = mybir.dt.float16

# "mixed": scores in f32r (x32 shipped), t stored f32r.  rel err ~9e-3.
# "f16":   scores in fp16 (no x32), t stored f16.        rel err ~1.25e-2.
SCHEME = "f16"


def _ceil_div(a, b):
    return (a + b - 1) // b


def build_nc(scheme=SCHEME):
    mixed = scheme == "mixed"
    nc = bacc.Bacc(None)

    # inputs ship pre-tiled from the host in exact SBUF layout
    # ([128 partitions, ...]) so every DMA line is fully contiguous
    # head = [wv half 0 | x16 chunk 0] combined: ONE 128-line DMA gates
    # the first V matmul group instead of two
    head_d = nc.declare_dram_parameter("head", [P, DT, 896], f16, isOutput=False)
    x16_d = nc.declare_dram_parameter(
        "x16", [SC - 1, P, DT, 512], f16, isOutput=False)
    m16_d = nc.declare_dram_parameter("m16", [P, DT, D], f16, isOutput=False)
    wv_d = nc.declare_dram_parameter("wv", [2, P, DT, 384], f16, isOutput=False)
    if mixed:
        x32_d = nc.declare_dram_parameter(
            "x32", [SC, P, DT, 512], f32r, isOutput=False)
    # partition-major output: host de-tiles [P, ST, H] -> [S, H]
    out_d = nc.declare_dram_parameter("out", [P, ST, H], f16, isOutput=True)

    t_dt = f32r if mixed else f16

    with tile.TileContext(nc, pool_alloc_mode="queue") as tc, ExitStack() as ctx:
        persist = ctx.enter_context(tc.tile_pool(name="persist", bufs=1))
        t_s = persist.tile([P, HT, S], t_dt)     # 48KB/part (24 if f16)
        V = persist.tile([P, ST, H], f16)        # 24KB/part
        ident16 = persist.tile([P, P], f16)
        # zcm = [512 zeros | 128-col causal mask]; slice [640-w:640] puts
        # the mask on the last 128 of a w-wide diag chunk, zeros elsewhere
        zcm = persist.tile([P, 640], f32)
        if mixed:
            x32_s = persist.tile([P, SC, DT, 512], f32r)  # 48KB/part

        p1pool = tc.alloc_tile_pool(name="p1", bufs=1)
        hpool = p1pool if mixed else persist
        hd_s = hpool.tile([P, DT, 896], f16)        # wv0 | x16 chunk 0
        x16t = hpool.tile([P, SC - 1, DT, 512], f16)  # x16 chunks 1..3
        m16_s = p1pool.tile([P, DT, D], f16)
        wv1_s = p1pool.tile([P, DT, 384], f16)

        def x16_s(sc, dt_, a, b):
            if sc == 0:
                return hd_s[:, dt_, 384 + a:384 + b]
            return x16t[:, sc - 1, dt_, a:b]

        # head-critical combined load first on the Sync queue: the first
        # V matmul group needs exactly this one transfer
        nc.sync.dma_start(out=hd_s, in_=head_d[:, :, :])
        # near-term loads on the Sync queue
        nc.sync.dma_start(out=wv1_s, in_=wv_d[1])
        for sc in range(1, SC):
            nc.sync.dma_start(out=x16t[:, sc - 1], in_=x16_d[sc - 1])

        # pre-ramp the PE clock (HAM releases the throttle after ~4us of
        # sustained activity) with junk matmuls while the head DMA lands
        with tc.tile_pool(name="warm", bufs=1) as warm_pool, \
             tc.tile_pool(name="warm_ps", bufs=1, space="PSUM") as warm_pp:
            junk = warm_pool.tile([P, 512], f16)
            nc.vector.memset(junk, 0.125)
            wps = warm_pp.tile([P, 512], f32, tag="wps", name="wps")
            for i in range(14):
                nc.tensor.matmul(
                    wps, junk[:, 0:P], junk,
                    start=(i == 0), stop=(i == 13))

        make_identity(nc, ident16)
        nc.gpsimd.memset(zcm[:, 0:512], 0.0)
        make_causal_mask(nc, zcm[:, 512:640], mask_val=1e10)

        # one [P,512]-bank PSUM pool spans phases 1b/1a and the phase-2
        # scores: no pool-close WAR barriers at the phase transitions
        pp = ctx.enter_context(
            tc.tile_pool(name="mm_ps", bufs=4, space="PSUM"))

        # ---- phase 1b: V = x16 (stationary) x Wv16 (moving) --------------
        if True:
            bulk_anchor = None
            for sc in range(SC):
                if sc == 1:
                    # bulk loads (x32 24KB/part + m16) stream on the SWDGE
                    # ring once the head-critical transfers are done
                    assert bulk_anchor is not None
                    bulk = [(m16_s, m16_d[:, :, :])]
                    if mixed:
                        bulk += [(x32_s[:, c], x32_d[c]) for c in range(SC)]
                    for dst, src in bulk:
                        dma = nc.gpsimd.dma_start(out=dst, in_=src)
                        bass_rust.add_dep_helper(
                            dma.ins, bulk_anchor.ins, sync=True,
                            reason="bulk load waits for first V chunk")
                for hc in range(2):
                    for sti in range(4):
                        off = sti * P
                        ps = pp.tile([P, 512], f32, tag="ps", name="ps")
                        for dt_ in range(DT):
                            nc.tensor.matmul(
                                ps[:, :384],
                                x16_s(sc, dt_, off, off + P),
                                hd_s[:, dt_, 0:384] if hc == 0
                                else wv1_s[:, dt_, :],
                                start=(dt_ == 0),
                                stop=(dt_ == DT - 1),
                            )
                        cp = nc.vector.tensor_copy(
                            V[:, sc * 4 + sti, hc * 384:(hc + 1) * 384],
                            ps[:, :384])
                        if sc == 0 and hc == 0 and sti == 3:
                            bulk_anchor = cp

        # ---- phase 1a: tT = M16 (stationary) x x16T (moving) -------------
        # reversed s-chunk order: phase 2 runs qt descending, so high-qt
        # tiles (which need the last t chunk for their stationary) unblock
        # right after the first chunk-group here
        if True:
            for sc in range(SC - 1, -1, -1):
                for ht in range(HT):
                    ps = pp.tile([P, 512], f32, tag="ps", name="ps")
                    for dt_ in range(DT):
                        nc.tensor.matmul(
                            ps,
                            m16_s[:, dt_, ht * P:(ht + 1) * P],
                            x16_s(sc, dt_, 0, 512),
                            start=(dt_ == 0),
                            stop=(dt_ == DT - 1),
                        )
                    nc.scalar.copy(t_s[:, ht, sc * 512:(sc + 1) * 512], ps)
        p1pool.release()
        if mixed:
            def xk_s(nch, dt_, a, b):
                return x32_s[:, nch, dt_, a:b]
        else:
            xk_s = x16_s  # scores k-side reads x16 chunks directly

        # ---- phase 2: attention ------------------------------------------
        with tc.tile_pool(name="p2_strip", bufs=4) as strip_pool, \
             tc.tile_pool(name="p2_exp", bufs=4) as exp_pool, \
             tc.tile_pool(name="p2_expT", bufs=4) as expT_pool, \
             tc.tile_pool(name="p2_stat", bufs=8) as stat_pool, \
             tc.tile_pool(name="p2_acc", bufs=2) as acc_pool, \
             tc.tile_pool(name="p2_ps_t", bufs=2, space="PSUM") as ps_t_pool, \
             tc.tile_pool(name="p2_ps_o", bufs=2, space="PSUM") as ps_o_pool:

            def softmax_part(qt, expT, rinv):
                """scores strip -> exp16 -> transposed exp blocks + 1/rowsum."""
                L = qt + 1
                cols = L * P
                nchn = _ceil_div(cols, 512)
                strip = strip_pool.tile([P, S], f32, tag="strip", name="strip")
                for nch in range(nchn):
                    w = min(512, cols - nch * 512)
                    # f32r needs moving >= 256 for full rate; pad short
                    # tails with throwaway columns
                    wp = max(w, 256) if mixed else w
                    ps = pp.tile([P, 512], f32, tag="ps", name="ps")
                    for dt_ in range(DT):
                        nc.tensor.matmul(
                            ps[:, :wp],
                            t_s[:, dt_, qt * P:(qt + 1) * P],
                            xk_s(nch, dt_, 0, wp),
                            start=(dt_ == 0),
                            stop=(dt_ == DT - 1),
                        )
                    # strip = mask - scores (masked slots become +1e10,
                    # transparent to the min-reduce below)
                    in1 = zcm[:, 640 - w:640] if nch == nchn - 1 else zcm[:, 0:w]
                    nc.vector.tensor_sub(
                        strip[:, nch * 512:nch * 512 + w], in1, ps[:, :w])
                nmax = stat_pool.tile([P, 1], f32, tag="nmax", name="nmax")
                nc.vector.tensor_reduce(
                    nmax, strip[:, :cols],
                    axis=mybir.AxisListType.X, op=mybir.AluOpType.min,
                )
                rsum = stat_pool.tile([P, 1], f32, tag="rsum", name="rsum")
                exp16 = exp_pool.tile([P, S], f16, tag="exp16", name="exp16")
                # exp(in*-1 + (-max)) = exp(scores + mask - max)
                nc.scalar.activation(
                    exp16[:, :cols], strip[:, :cols],
                    mybir.ActivationFunctionType.Exp,
                    bias=nmax, scale=-1.0, accum_out=rsum,
                )
                nc.vector.reciprocal(rinv, rsum)
                for j0 in range(0, L, 4):
                    jn = min(4, L - j0)
                    pst = ps_t_pool.tile([P, 512], f16, tag="ps_t", name="ps_t")
                    for i in range(jn):
                        nc.tensor.transpose(
                            pst[:, i * P:(i + 1) * P],
                            exp16[:, (j0 + i) * P:(j0 + i + 1) * P],
                            ident16)
                    nc.scalar.copy(expT[:, j0:j0 + jn, :], pst[:, :jn * P])

            def attnv_part(qt, expT, rinv, acc):
                for hc in range(2):
                    pso = ps_o_pool.tile([P, 384], f32, tag="ps_o", name="ps_o")
                    for j in range(qt + 1):
                        nc.tensor.matmul(
                            pso,
                            expT[:, j, :],
                            V[:, j, hc * 384:(hc + 1) * 384],
                            start=(j == 0),
                            stop=(j == qt),
                        )
                    nc.vector.tensor_scalar_mul(
                        acc[:, qt % 4, hc * 384:(hc + 1) * 384], pso, rinv)

            acc = None

            def full_tile(qt):
                nonlocal acc
                if qt % 4 == 3:
                    acc = acc_pool.tile([P, 4, H], f16, tag="acc", name="acc")
                expT = expT_pool.tile([P, ST, P], f16, tag="expT", name="expT")
                rinv = stat_pool.tile([P, 1], f32, tag="rinv", name="rinv")
                softmax_part(qt, expT, rinv)
                attnv_part(qt, expT, rinv, acc)
                if qt % 4 == 0:
                    nc.sync.dma_start(out=out_d[:, qt:qt + 4, :], in_=acc)

            for qt in range(ST - 1, 6, -1):   # 15..7
                full_tile(qt)
            # small tiles (qt<=3): emit their softmax NOW so the serial
            # strip->exp->transpose chains hide under qt 6..4's PE work;
            # their attn@V runs at the very end with everything ready.
            # Dedicated (non-rotating) expT/rinv tiles avoid queue-pool
            # WAR deps from later tiles.
            sm = []
            for i in range(4):
                expT_sm = persist.tile([P, 4, P], f16, name=f"expT_sm{i}")
                rinv_sm = persist.tile([P, 1], f32, name=f"rinv_sm{i}")
                sm.append((expT_sm, rinv_sm))
            for qt in (3, 2, 1, 0):
                softmax_part(qt, sm[qt][0], sm[qt][1])
            for qt in (6, 5, 4):
                full_tile(qt)
            acc = acc_pool.tile([P, 4, H], f16, tag="acc", name="acc")
            for qt in (3, 2, 1, 0):
                attnv_part(qt, sm[qt][0], sm[qt][1], acc)
            nc.sync.dma_start(out=out_d[:, 0:4, :], in_=acc)

    nc.finalize()
    return nc


_NC_CACHE = None


def _get_nc():
    global _NC_CACHE
    if _NC_CACHE is None:
        _NC_CACHE = build_nc()
    return _NC_CACHE


def _tile_rows(a):
    """[D, N] -> [128, D//128, N] (partition-major SBUF layout)."""
    d, n = a.shape
    return np.ascontiguousarray(a.reshape(d // P, P, n).transpose(1, 0, 2))


def make_in_maps(x, Wq, Wk, Wv):
    M = (Wq.astype(np.float64) @ Wk.astype(np.float64).T).astype(np.float32)
    m16 = _tile_rows(M.astype(np.float16))
    wv16 = _tile_rows(Wv.astype(np.float16))
    wv16 = np.ascontiguousarray(
        wv16.reshape(P, DT, 2, 384).transpose(2, 0, 1, 3))
    mixed = SCHEME == "mixed"
    in_maps = []
    for c in range(N_CORES):
        xT = np.ascontiguousarray(x[c].T)  # [D, S] f32
        xc32 = np.ascontiguousarray(
            xT.reshape(DT, P, SC, 512).transpose(2, 1, 0, 3))
        xc16 = xc32.astype(np.float16)
        head = np.ascontiguousarray(
            np.concatenate([wv16[0], xc16[0]], axis=-1))
        im = {"head": head, "x16": np.ascontiguousarray(xc16[1:]),
              "m16": m16, "wv": wv16}
        if mixed:
            im["x32"] = xc32
        in_maps.append(im)
    return in_maps


def kernel(x, Wq, Wk, Wv):
    x = np.asarray(x, dtype=np.float32)
    Wq = np.asarray(Wq, dtype=np.float32)
    Wk = np.asarray(Wk, dtype=np.float32)
    Wv = np.asarray(Wv, dtype=np.float32)

    nc = _get_nc()
    in_maps = make_in_maps(x, Wq, Wk, Wv)
    res = run_bass_kernel_spmd(nc, in_maps, list(range(N_CORES)))
    out = np.stack(
        [res.results[c]["out"].transpose(1, 0, 2).reshape(S, H)
         for c in range(N_CORES)], axis=0)
    return out.astype(np.float32)
